# revision 14
# baseline (speedup 1.0000x reference)
"""AllPoleDigitalFilter Trainium2 kernel — segmented block-solve, v8.

y[t] = K_int[t]*x[t] - sum_{i=1..30} a_int[t,i] * y[t-i]
with a_int/K_int linearly interpolated from frame coefficients (period 80).

Per core (8 of 64 sequences): overlap-save into 128 chunk instances
(2 parities x 8 seqs x 8 chunks) of 1080 samples (80-sample warmup from
zero state).

Vector engine advances the recurrence S=8 samples per 4 instructions via
a custom DVE op CUMSUM_MUL (out = running cumsum of Src0*Src1):
  far:  cumsum over afull[t0+k,d]*ybuf[t0+k+d] (in-segment slots still
        zero); 0-stride out keeps row-final cumsums -> fscr[1..8]
  b[k] = xgf[t0+k] - (fscr[k+1]-fscr[k])
  near: cumsum over G[seg]*b with G = row-differenced E = (I+N)^{-1};
        telescoping makes row-final cumsums equal y; 0-stride out
        writes ybuf directly.
E rows are built by a second custom op CUMSUM_NEGMUL (cumsum of
-Src0*Src1) per (k, column): batched over segments with a 0-stride
output, no scratch needed. E starts from a host-shipped identity table.

Engine placement (measured: ANY Pool/gpsimd compute running concurrently
starves the Vector engine ~10x, so Pool does nothing but two memsets):
 - Vector: chain, E build, G diff, gain channel, interp for the first
   two column blocks, frame-term adds (bf16 2x) from staged slices.
 - Scalar/ACT: interpolation fraction-multiply for the two big column
   blocks as per-phase activation Copy ops (scale = per-partition
   fraction), baseline-style; plus half the DMA issues.
 - The frame term comes from a host-side replicated gather (pure
   layout), streamed through SBUF staging buffers.
"""
import numpy as np

B, T = 64, 16000
NSEQ = 8
NCORE = 8
W = 80
L = 1000
WP = W + L         # 1080
S = 8
NSEG = WP // S     # 135
NU = WP // 40      # 27 half-frames
XP_LEN = W + T

BLK_U = [3, 6, 9, 9]         # blocks in half-frames (sum = 27)
DVE_BLOCKS = (0, 1)          # interp-mult on DVE for these, ACT for rest
_prog = None
_ops = None


def _register_ops():
    """Append CUMSUM_MUL / CUMSUM_NEGMUL to the custom-DVE registry
    (documented extension point; per-NEFF table, existing ops kept)."""
    global _ops
    if _ops is not None:
        return _ops
    from concourse.dve_spec import Spec, Src0, Src1, Zero, scan, AluOp, \
        lower, _has_src1
    from concourse.dve_uop import DveOpSpec
    from concourse.dve_ops import DveOp, OPS, _SUB_OPCODE_FOR_NAME, \
        _CUSTOM_DVE_ROW_BASE

    def reg(name, spec):
        if name in _SUB_OPCODE_FOR_NAME:
            return next(o for o in OPS if o.name == name)
        shas = {}
        for ver in ("v3", "v4"):
            s = DveOpSpec(name=name, opcode=0,
                          uops=lower(spec, ver=ver), rd1_en=_has_src1(spec))
            shas[ver] = s.sha(ver)
        op = DveOp(name, spec, subdim=False, uops_sha=shas)
        OPS.append(op)
        _SUB_OPCODE_FOR_NAME[name] = _CUSTOM_DVE_ROW_BASE + len(OPS) - 1
        return op

    cm = reg("CUMSUM_MUL_APDF", Spec(
        body=scan(AluOp.ADD, Src0 * Src1),
        reference=lambda in0, in1, c0, c1, c2: np.cumsum(
            in0.astype(np.float32) * in1.astype(np.float32), axis=-1)))
    cn = reg("CUMSUM_NEGMUL_APDF", Spec(
        body=scan(AluOp.ADD, Zero - Src0 * Src1),
        reference=lambda in0, in1, c0, c1, c2: np.cumsum(
            -(in0.astype(np.float32) * in1.astype(np.float32)), axis=-1)))
    _ops = (cm, cn)
    return _ops


def _build_program():
    import concourse.bacc as bacc
    import concourse.mybir as mybir
    import concourse.bass as bass
    from concourse.tile import TileContext

    CM, CN = _register_ops()
    import resetscan
    RM, RN = resetscan.register_reset_ops()

    f32 = mybir.dt.float32
    bf16 = mybir.dt.bfloat16
    AP = bass.AP
    mult = mybir.AluOpType.mult
    add = mybir.AluOpType.add
    sub = mybir.AluOpType.subtract

    nc = bacc.Bacc("TRN2", target_bir_lowering=False, name="apdf8",
                   detect_race_conditions=False)
    xp_d = nc.dram_tensor("xp", (NSEQ, XP_LEN), f32, kind="ExternalInput")
    frhr_d = nc.dram_tensor("frhr", (128, NU, 30), f32, kind="ExternalInput")
    frh1r_d = nc.dram_tensor("frh1r", (128, NU, 30), f32, kind="ExternalInput")
    kfr_d = nc.dram_tensor("kfr", (128, NU), f32, kind="ExternalInput")
    kfr1_d = nc.dram_tensor("kfr1", (128, NU), f32, kind="ExternalInput")
    ftab_d = nc.dram_tensor("ftab", (128, WP), f32, kind="ExternalInput")
    ftabp_d = nc.dram_tensor("ftabp", (128, 80), f32, kind="ExternalInput")
    frhrep_d = nc.dram_tensor("frhrep", (128, WP, 30), bf16,
                              kind="ExternalInput")
    y_d = nc.dram_tensor("y", (NSEQ, T), f32, kind="ExternalOutput")

    with TileContext(nc) as tc:
        with tc.tile_pool(name="sbuf", bufs=1) as pool:
            afull = pool.tile([128, WP, 30], bf16)
            E = pool.tile([128, NSEG, 64], f32)
            ybuf = pool.tile([128, 30 + WP], f32)
            xwin = pool.tile([128, WP], f32)
            xgf = pool.tile([128, WP], f32)
            kt = pool.tile([128, WP], f32)
            ftab = pool.tile([128, WP], f32)
            ftabp = pool.tile([128, 80], f32)
            frba = pool.tile([128, 10800], bf16)
            frbb = pool.tile([128, 10800], bf16)
            frhr = pool.tile([128, NU, 30], f32)
            frh1r = pool.tile([128, NU, 30], f32)
            dfhr = pool.tile([128, NU, 30], f32)
            kfr = pool.tile([128, NU], f32)
            kfr1 = pool.tile([128, NU], f32)
            dk = pool.tile([128, NU], f32)
            fscr = pool.tile([128, 9], f32)
            bseg = pool.tile([128, S], f32)

            def tap(t, off, apl):
                base = t[:]
                return AP(tensor=base.tensor, offset=off, ap=apl)

            AF = WP * 30
            ES = NSEG * 64
            YS = 30 + WP

            # ---------------- input DMAs (sync queue) ----------------
            nc.sync.dma_start(out=frhr[:].rearrange("p u d -> p (u d)"),
                              in_=frhr_d[:].rearrange("p u d -> p (u d)"))
            nc.sync.dma_start(out=frh1r[:].rearrange("p u d -> p (u d)"),
                              in_=frh1r_d[:].rearrange("p u d -> p (u d)"))
            nc.sync.dma_start(out=kfr[:], in_=kfr_d[:])
            nc.sync.dma_start(out=kfr1[:], in_=kfr1_d[:])
            nc.sync.dma_start(out=ftab[:], in_=ftab_d[:])
            nc.scalar.dma_start(out=ftabp[:], in_=ftabp_d[:])

            xw4 = xwin[:].rearrange("(c s k) j -> c s k j", c=2, s=8, k=8)
            for par in (0, 1):
                for s in range(NSEQ):
                    xsrc = AP(tensor=xp_d, offset=s * XP_LEN + 1000 * par,
                              ap=[[2000, 8], [1, WP]])
                    nc.sync.dma_start(out=xw4[par, s], in_=xsrc)

            # ---------------- init ----------------
            nc.gpsimd.memset(ybuf[:], 0.0)
            nc.gpsimd.memset(fscr[:], 0.0)
            nc.gpsimd.memset(E[:].rearrange("p s e -> p (s e)"), 0.0)
            nc.gpsimd.memset(
                tap(E, 0, [[ES, 128], [64, NSEG], [9, 8]]), 1.0)

            nc.vector.tensor_tensor(
                out=dfhr[:].rearrange("p u d -> p (u d)"),
                in0=frh1r[:].rearrange("p u d -> p (u d)"),
                in1=frhr[:].rearrange("p u d -> p (u d)"), op=sub)
            nc.vector.tensor_tensor(out=dk[:], in0=kfr1[:], in1=kfr[:], op=sub)

            # gain channel, whole window (DVE, ~4us)
            nc.vector.tensor_tensor(
                out=tap(kt, 0, [[WP, 128], [40, NU], [1, 40]]),
                in0=tap(dk, 0, [[NU, 128], [1, NU], [0, 40]]),
                in1=tap(ftab, 0, [[WP, 128], [40, NU], [1, 40]]),
                op=mult)
            nc.vector.tensor_tensor(
                out=tap(kt, 0, [[WP, 128], [40, NU], [1, 40]]),
                in0=tap(kt, 0, [[WP, 128], [40, NU], [1, 40]]),
                in1=tap(kfr, 0, [[NU, 128], [1, NU], [0, 40]]),
                op=add)
            nc.vector.tensor_tensor(out=xgf[:], in0=kt[:], in1=xwin[:],
                                    op=mult)

            # -------- block table --------
            blocks = []
            u0 = 0
            s0 = 0
            for ublk in BLK_U:
                blocks.append((u0, ublk, s0, ublk * 40 // S))
                u0 += ublk
                s0 += ublk * 40 // S

            frbi = [0]

            def emit_interp_mult(bi):
                """afull[:, block, :] = dfhr(u) * frac.  DVE for the small
                leading blocks, ACT per-phase ops for the big ones."""
                u0, ublk, s0, segb = blocks[bi]
                c0 = u0 * 40
                if bi in DVE_BLOCKS:
                    nc.vector.tensor_tensor(
                        out=tap(afull, c0 * 30,
                                [[AF, 128], [1200, ublk], [30, 40], [1, 30]]),
                        in0=tap(dfhr, u0 * 30,
                                [[NU * 30, 128], [30, ublk], [0, 40], [1, 30]]),
                        in1=tap(ftab, c0,
                                [[WP, 128], [40, ublk], [1, 40], [0, 30]]),
                        op=mult)
                    return
                for par in (0, 1):
                    us = [u for u in range(u0, u0 + ublk) if u % 2 == par]
                    if not us:
                        continue
                    uq = us[0]
                    nq = len(us)
                    for r in range(40):
                        nc.scalar.activation(
                            out=tap(afull, (40 * uq + r) * 30,
                                    [[AF, 128], [2400, nq], [1, 30]]),
                            in_=tap(dfhr, uq * 30,
                                    [[NU * 30, 128], [60, nq], [1, 30]]),
                            func=mybir.ActivationFunctionType.Copy,
                            bias=0.0,
                            scale=ftabp[:, 40 * par + r : 40 * par + r + 1])

            def emit_frb_dmas(bi):
                u0, ublk, s0, segb = blocks[bi]
                c0 = u0 * 40
                nel = ublk * 40 * 30
                buf = frba if (frbi[0] % 2 == 0) else frbb
                frbi[0] += 1
                chunks = []
                eng = nc.scalar if bi in DVE_BLOCKS else nc.sync
                for ci in range((nel + 4799) // 4800):
                    q0 = ci * 4800
                    qn = min(4800, nel - q0)
                    eng.dma_start(
                        out=buf[:, q0 : q0 + qn],
                        in_=AP(tensor=frhrep_d, offset=c0 * 30 + q0,
                               ap=[[AF, 128], [1, qn]]))
                    chunks.append((q0, qn, buf))
                return chunks

            def emit_add(bi, chunk):
                u0, ublk, s0, segb = blocks[bi]
                c0 = u0 * 40
                q0, qn, buf = chunk
                if bi in DVE_BLOCKS:
                    nc.vector.tensor_tensor(
                        out=tap(afull, c0 * 30 + q0, [[AF, 128], [1, qn]]),
                        in0=tap(afull, c0 * 30 + q0, [[AF, 128], [1, qn]]),
                        in1=buf[:, q0 : q0 + qn], op=add)
                    return
                # accumulate-DMA (gpsimd queue; <=1800 elems per issue)
                for p0 in range(0, qn, 1800):
                    pn = min(1800, qn - p0)
                    nc.gpsimd.dma_start(
                        out=tap(afull, c0 * 30 + q0 + p0,
                                [[AF, 128], [1, pn]]),
                        in_=buf[:, q0 + p0 : q0 + p0 + pn],
                        accum_op=add)

            def emit_e_kj(bi, kk, jj):
                u0, ublk, s0, segb = blocks[bi]
                nc.vector._custom_dve(
                    RN,
                    out=tap(E, s0 * 64 + kk * 8 + jj,
                            [[ES, 128], [64, segb], [0, kk]]),
                    in0=tap(afull, (s0 * S + kk) * 30 + 29,
                            [[AF, 128], [240, segb], [-1, kk]]),
                    in1=tap(E, s0 * 64 + (kk - 1) * 8 + jj,
                            [[ES, 128], [64, segb], [-8, kk]]),
                )

            def emit_e(bi):
                for kk in range(1, S):
                    for jj in range(kk):
                        emit_e_kj(bi, kk, jj)

            def emit_slab1():
                yva = ybuf[:, 30 + W : 30 + W + 500].rearrange(
                    "(c s k) j -> c s k j", c=2, s=8, k=8)
                for par in (0, 1):
                    for s in range(NSEQ):
                        dst = AP(tensor=y_d, offset=s * T + 1000 * par,
                                 ap=[[2000, 8], [1, 500]])
                        nc.sync.dma_start(out=dst, in_=yva[par, s])

            def emit_chain(bi, interleave):
                u0, ublk, s0, segb = blocks[bi]
                for sl in range(segb):
                    seg = s0 + sl
                    t0 = seg * S
                    nc.vector._custom_dve(
                        RM,
                        out=tap(fscr, 1, [[9, 128], [1, S], [0, 30]]),
                        in0=tap(afull, t0 * 30,
                                [[AF, 128], [30, S], [1, 30]]),
                        in1=tap(ybuf, t0, [[YS, 128], [1, S], [1, 30]]),
                    )
                    nc.vector.tensor_tensor(
                        out=bseg[:], in0=xgf[:, t0 : t0 + S],
                        in1=fscr[:, 1 : 1 + S], op=sub)
                    nc.vector._custom_dve(
                        RM,
                        out=tap(ybuf, 30 + t0, [[YS, 128], [1, S], [0, S]]),
                        in0=tap(E, seg * 64, [[ES, 128], [8, S], [1, S]]),
                        in1=tap(bseg, 0, [[S, 128], [0, S], [1, S]]),
                    )
                    if seg == 77:
                        emit_slab1()
                    if seg == 128:
                        yvb = ybuf[:, 30 + W + 500 : 30 + W + 850].rearrange(
                            "(c s k) j -> c s k j", c=2, s=8, k=8)
                        for par in (0, 1):
                            for s in range(NSEQ):
                                dst = AP(tensor=y_d,
                                         offset=s * T + 1000 * par + 500,
                                         ap=[[2000, 8], [1, 350]])
                                eng = nc.scalar if (s % 2 == 0) else nc.sync
                                eng.dma_start(out=dst, in_=yvb[par, s])
                    for th in interleave.get(sl, ()):
                        th()

            # ACT interp for the big blocks (must be emitted after dfhr
            # so the dependency is tracked)
            for bi in range(len(blocks)):
                if bi not in DVE_BLOCKS:
                    emit_interp_mult(bi)

            # -------- block 0 prefix on DVE --------
            emit_interp_mult(0)
            for ch in emit_frb_dmas(0):
                emit_add(0, ch)
            emit_e(0)

            # -------- pipelined chain; DVE prep of block nxt interleaved
            # into chain(bi) --------
            for bi in range(len(blocks)):
                nxt = bi + 1
                interleave = {}
                if nxt < len(blocks):
                    segb = blocks[bi][3]
                    chunks = emit_frb_dmas(nxt)
                    pos = max(1, int(segb * 0.45))
                    th = []
                    if nxt in DVE_BLOCKS:
                        th.append(lambda b=nxt: emit_interp_mult(b))
                    th += [lambda c=ch: emit_add(nxt, c) for ch in chunks]
                    interleave.setdefault(pos, []).extend(th)
                    p2 = max(pos + 1, int(segb * 0.55))
                    kjs = [(kk, jj) for kk in range(1, S) for jj in range(kk)]
                    span = max(1, segb - 1 - p2)
                    for i, (kk, jj) in enumerate(kjs):
                        p = min(segb - 1, p2 + (i * span) // len(kjs))
                        interleave.setdefault(p, []).append(
                            lambda b=nxt, k=kk, q=jj: emit_e_kj(b, k, q))
                emit_chain(bi, interleave)

            # ---------------- final output DMAs ----------------
            yv = ybuf[:, 30 + W + 850 : 30 + W + L].rearrange(
                "(c s k) j -> c s k j", c=2, s=8, k=8)
            for par in (0, 1):
                for s in range(NSEQ):
                    dst = AP(tensor=y_d, offset=s * T + 1000 * par + 850,
                             ap=[[2000, 8], [1, 150]])
                    eng = nc.scalar if (s % 2 == 0) else nc.sync
                    eng.dma_start(out=dst, in_=yv[par, s])

    nc.compile()
    return nc


def _get_prog():
    global _prog
    if _prog is None:
        _prog = _build_program()
    return _prog


def _host_inputs(x, a):
    import ml_dtypes

    x = np.ascontiguousarray(x, dtype=np.float32)
    a = np.ascontiguousarray(a, dtype=np.float32)
    xp = np.zeros((B, XP_LEN), np.float32)
    xp[:, W:] = x
    af = np.concatenate([a, a[:, -1:, :]], axis=1)   # [B, 201, 31]
    nfr = af.shape[1]
    par = np.arange(128) // 64
    sq = (np.arange(128) % 64) // 8
    kc = np.arange(128) % 8
    m = 2 * kc + par
    w0 = 1000 * m - W
    n0 = np.floor_divide(w0, 80)
    phi = w0 - 80 * n0              # 0 or 40
    u = np.arange(NU)
    nl = (40 * u[None, :] + phi[:, None]) // 80
    idx = np.clip(n0[:, None] + nl, 0, nfr - 1)
    idx1 = np.clip(n0[:, None] + nl + 1, 0, nfr - 1)
    jl = np.arange(WP)
    ftab = (((jl[None, :] + phi[:, None]) % 80) / 80.0).astype(np.float32)
    rr = np.arange(80)
    ftabp = (((rr[None, :] + phi[:, None]) % 80) / 80.0).astype(np.float32)
    rev = 30 - np.arange(30)
    in_maps = []
    for c in range(NCORE):
        sl = slice(c * NSEQ, (c + 1) * NSEQ)
        seqg = c * NSEQ + sq
        frhr = af[seqg[:, None, None], idx[:, :, None], rev[None, None, :]]
        frh1r = af[seqg[:, None, None], idx1[:, :, None], rev[None, None, :]]
        kfr = af[seqg[:, None], idx, 0]
        kfr1 = af[seqg[:, None], idx1, 0]
        in_maps.append({
            "xp": xp[sl],
            "frhr": np.ascontiguousarray(frhr, np.float32),
            "frh1r": np.ascontiguousarray(frh1r, np.float32),
            "kfr": np.ascontiguousarray(kfr, np.float32),
            "kfr1": np.ascontiguousarray(kfr1, np.float32),
            "ftab": ftab,
            "ftabp": ftabp,
            "frhrep": np.ascontiguousarray(
                np.repeat(frhr, 40, axis=1).astype(
                    ml_dtypes.bfloat16)).view(np.uint16),
        })
    return in_maps


def kernel(x, a):
    from concourse import bass_utils

    nc = _get_prog()
    in_maps = _host_inputs(x, a)
    res = bass_utils.run_bass_kernel_spmd(nc, in_maps, core_ids=list(range(NCORE)))
    out = np.empty((B, T), np.float32)
    for c in range(NCORE):
        out[c * NSEQ : (c + 1) * NSEQ] = res.results[c]["y"]
    return out


# revision 15
# speedup vs baseline: 1.0874x; 1.0874x over previous
"""AllPoleDigitalFilter Trainium2 kernel — segmented block-solve, v14.

y[t] = K_int[t]*x[t] - sum_{i=1..30} a_int[t,i] * y[t-i]
with a_int/K_int linearly interpolated from frame coefficients (period 80).

Per core (8 of 64 sequences): overlap-save into 128 chunk instances
(2 parities x 8 seqs x 8 chunks) of 1080 samples (80-sample warmup from
zero state).

The Vector engine advances the recurrence S=8 samples in just TWO
custom-DVE instructions (per-page-reset cumsum ops, registered at
runtime into the per-NEFF table):
  far:  page-reset cumsum over the [8 x 31] rectangle
        afull[t0+k,d] * ybuf[t0+k+d]. Column d=30 of afull holds the
        NEGATED gain-modulated input -K_int*x and its ybuf partner (the
        still-unwritten current-sample slot) is prefilled 1.0, while the
        in-segment coefficient cells of afull are zeroed after the E
        build reads them — so each page-final value is exactly
        sum_far a*y - xg = -b[k], captured compactly by a 0-stride
        output into fscr[1..8].
  near: page-reset negated cumsum over E[seg][8x8] * (-b) broadcast,
        0-stride output writing y straight into ybuf (overwriting the
        1.0 prefills).
E = (I+N)^{-1} per segment is built on-device by the same negated
reset-cumsum op, one instruction per (row k, column j), batched over a
block's segments.

Engine placement (measured: concurrent Pool/gpsimd COMPUTE starves the
Vector engine ~10x; pure DMA issue is benign): Vector does the chain,
E build, interpolation for the two leading column blocks, frame-term
adds (bf16 2x) and the gain channel; ACT does the big column blocks'
fraction-multiply as per-phase activation Copies; gpsimd only runs
startup memsets; HWDGE queues stream a host-side replicated frame-term
gather (pure layout) through SBUF staging.
"""
import numpy as np

B, T = 64, 16000
NSEQ = 8
NCORE = 8
W = 80
L = 1000
WP = W + L         # 1080
S = 8
NSEG = WP // S     # 135
NU = WP // 40      # 27 half-frames
XP_LEN = W + T

BLK_U = [3, 6, 9, 9]         # blocks in half-frames (sum = 27)
DVE_BLOCKS = (0, 1)          # interp-mult on DVE for these, ACT for rest
_prog = None


def _build_program():
    import concourse.bacc as bacc
    import concourse.mybir as mybir
    import concourse.bass as bass
    from concourse.tile import TileContext
    import resetscan

    RM, RN = resetscan.register_reset_ops()

    f32 = mybir.dt.float32
    bf16 = mybir.dt.bfloat16
    AP = bass.AP
    mult = mybir.AluOpType.mult
    add = mybir.AluOpType.add
    sub = mybir.AluOpType.subtract

    nc = bacc.Bacc("TRN2", target_bir_lowering=False, name="apdf14",
                   detect_race_conditions=False)
    xp_d = nc.dram_tensor("xp", (NSEQ, XP_LEN), f32, kind="ExternalInput")
    frhr_d = nc.dram_tensor("frhr", (128, NU, 30), f32, kind="ExternalInput")
    frh1r_d = nc.dram_tensor("frh1r", (128, NU, 30), f32, kind="ExternalInput")
    kfr_d = nc.dram_tensor("kfr", (128, NU), f32, kind="ExternalInput")
    kfr1_d = nc.dram_tensor("kfr1", (128, NU), f32, kind="ExternalInput")
    ftab_d = nc.dram_tensor("ftab", (128, WP), f32, kind="ExternalInput")
    ftabp_d = nc.dram_tensor("ftabp", (128, 80), f32, kind="ExternalInput")
    frhrep_d = nc.dram_tensor("frhrep", (128, WP, 30), bf16,
                              kind="ExternalInput")
    y_d = nc.dram_tensor("y", (NSEQ, T), f32, kind="ExternalOutput")

    with TileContext(nc) as tc:
        with tc.tile_pool(name="sbuf", bufs=1) as pool:
            afull = pool.tile([128, WP, 31], bf16)
            E = pool.tile([128, NSEG, 64], f32)
            ybuf = pool.tile([128, 30 + WP], f32)
            xwin = pool.tile([128, WP], f32)
            ktn = pool.tile([128, WP], f32)
            ftab = pool.tile([128, WP], f32)
            ftabp = pool.tile([128, 80], f32)
            frba = pool.tile([128, 10800], bf16)
            frbb = pool.tile([128, 10800], bf16)
            frhr = pool.tile([128, NU, 30], f32)
            frh1r = pool.tile([128, NU, 30], f32)
            dfhr = pool.tile([128, NU, 30], f32)
            kfr = pool.tile([128, NU], f32)
            kfr1 = pool.tile([128, NU], f32)
            dkn = pool.tile([128, NU], f32)
            fscr = pool.tile([128, 9], f32)

            def tap(t, off, apl):
                base = t[:]
                return AP(tensor=base.tensor, offset=off, ap=apl)

            AF = WP * 31     # afull partition stride (elements)
            ES = NSEG * 64
            YS = 30 + WP

            # ---------------- input DMAs ----------------
            nc.sync.dma_start(out=frhr[:].rearrange("p u d -> p (u d)"),
                              in_=frhr_d[:].rearrange("p u d -> p (u d)"))
            nc.sync.dma_start(out=frh1r[:].rearrange("p u d -> p (u d)"),
                              in_=frh1r_d[:].rearrange("p u d -> p (u d)"))
            nc.sync.dma_start(out=kfr[:], in_=kfr_d[:])
            nc.sync.dma_start(out=kfr1[:], in_=kfr1_d[:])
            nc.sync.dma_start(out=ftab[:], in_=ftab_d[:])
            nc.scalar.dma_start(out=ftabp[:], in_=ftabp_d[:])

            xw4 = xwin[:].rearrange("(c s k) j -> c s k j", c=2, s=8, k=8)
            for par in (0, 1):
                for s in range(NSEQ):
                    xsrc = AP(tensor=xp_d, offset=s * XP_LEN + 1000 * par,
                              ap=[[2000, 8], [1, WP]])
                    nc.sync.dma_start(out=xw4[par, s], in_=xsrc)

            # ---------------- init ----------------
            nc.gpsimd.memset(ybuf[:, 0:30], 0.0)
            nc.gpsimd.memset(ybuf[:, 30:], 1.0)
            nc.gpsimd.memset(E[:].rearrange("p s e -> p (s e)"), 0.0)
            nc.gpsimd.memset(
                tap(E, 0, [[ES, 128], [64, NSEG], [9, 8]]), 1.0)

            nc.vector.tensor_tensor(
                out=dfhr[:].rearrange("p u d -> p (u d)"),
                in0=frh1r[:].rearrange("p u d -> p (u d)"),
                in1=frhr[:].rearrange("p u d -> p (u d)"), op=sub)
            # negated gain delta: dkn = kfr - kfr1
            nc.vector.tensor_tensor(out=dkn[:], in0=kfr[:], in1=kfr1[:],
                                    op=sub)

            # gain channel: ktn = dkn*ftab - kfr = -(K_int frame interp);
            # afull[:, :, 30] = ktn * xwin = -K_int*x
            nc.vector.tensor_tensor(
                out=tap(ktn, 0, [[WP, 128], [40, NU], [1, 40]]),
                in0=tap(dkn, 0, [[NU, 128], [1, NU], [0, 40]]),
                in1=tap(ftab, 0, [[WP, 128], [40, NU], [1, 40]]),
                op=mult)
            nc.vector.tensor_tensor(
                out=tap(ktn, 0, [[WP, 128], [40, NU], [1, 40]]),
                in0=tap(ktn, 0, [[WP, 128], [40, NU], [1, 40]]),
                in1=tap(kfr, 0, [[NU, 128], [1, NU], [0, 40]]),
                op=sub)
            nc.vector.tensor_tensor(
                out=tap(afull, 30, [[AF, 128], [31, WP]]),
                in0=ktn[:], in1=xwin[:], op=mult)

            # -------- block table --------
            blocks = []
            u0 = 0
            s0 = 0
            for ublk in BLK_U:
                blocks.append((u0, ublk, s0, ublk * 40 // S))
                u0 += ublk
                s0 += ublk * 40 // S

            frbi = [0]

            def emit_interp_mult(bi):
                u0, ublk, s0, segb = blocks[bi]
                c0 = u0 * 40
                if bi in DVE_BLOCKS:
                    nc.vector.tensor_tensor(
                        out=tap(afull, c0 * 31,
                                [[AF, 128], [1240, ublk], [31, 40], [1, 30]]),
                        in0=tap(dfhr, u0 * 30,
                                [[NU * 30, 128], [30, ublk], [0, 40], [1, 30]]),
                        in1=tap(ftab, c0,
                                [[WP, 128], [40, ublk], [1, 40], [0, 30]]),
                        op=mult)
                    return
                for par in (0, 1):
                    us = [u for u in range(u0, u0 + ublk) if u % 2 == par]
                    if not us:
                        continue
                    uq = us[0]
                    nq = len(us)
                    for r in range(40):
                        nc.scalar.activation(
                            out=tap(afull, (40 * uq + r) * 31,
                                    [[AF, 128], [2480, nq], [1, 30]]),
                            in_=tap(dfhr, uq * 30,
                                    [[NU * 30, 128], [60, nq], [1, 30]]),
                            func=mybir.ActivationFunctionType.Copy,
                            bias=0.0,
                            scale=ftabp[:, 40 * par + r : 40 * par + r + 1])

            def emit_frb_dmas(bi):
                u0, ublk, s0, segb = blocks[bi]
                r0 = u0 * 40
                nrows = ublk * 40
                buf = frba if (frbi[0] % 2 == 0) else frbb
                frbi[0] += 1
                eng = nc.scalar if bi in DVE_BLOCKS else nc.sync
                chunks = []
                for ci in range(0, nrows, 160):
                    nr = min(160, nrows - ci)
                    eng.dma_start(
                        out=buf[:, ci * 30 : (ci + nr) * 30],
                        in_=AP(tensor=frhrep_d, offset=(r0 + ci) * 30,
                               ap=[[WP * 30, 128], [1, nr * 30]]))
                    chunks.append((ci, nr, buf))
                return chunks

            def emit_add(bi, chunk):
                """Frame-term add on DVE: strided afull rows += staged
                contiguous rows (bf16, packed innermost)."""
                u0, ublk, s0, segb = blocks[bi]
                r0 = u0 * 40
                ci, nr, buf = chunk
                nc.vector.tensor_tensor(
                    out=tap(afull, (r0 + ci) * 31,
                            [[AF, 128], [31, nr], [1, 30]]),
                    in0=tap(afull, (r0 + ci) * 31,
                            [[AF, 128], [31, nr], [1, 30]]),
                    in1=tap(buf, ci * 30, [[10800, 128], [30, nr], [1, 30]]),
                    op=add)

            def emit_e_kj(bi, kk, jj):
                u0, ublk, s0, segb = blocks[bi]
                nc.vector._custom_dve(
                    RN,
                    out=tap(E, s0 * 64 + kk * 8 + jj,
                            [[ES, 128], [64, segb], [0, kk]]),
                    in0=tap(afull, (s0 * S + kk) * 31 + 29,
                            [[AF, 128], [248, segb], [-1, kk]]),
                    in1=tap(E, s0 * 64 + (kk - 1) * 8 + jj,
                            [[ES, 128], [64, segb], [-8, kk]]),
                )

            def emit_e(bi):
                for kk in range(1, S):
                    for jj in range(kk):
                        emit_e_kj(bi, kk, jj)

            def emit_tri_zero(bi):
                """Zero the in-segment coefficient cells (row k, d>=30-k)
                after the E build has consumed them."""
                u0, ublk, s0, segb = blocks[bi]
                for kk in range(1, S):
                    nc.vector.tensor_scalar_mul(
                        tap(afull, (s0 * S + kk) * 31 + 30 - kk,
                            [[AF, 128], [248, segb], [1, kk]]),
                        tap(afull, (s0 * S + kk) * 31 + 30 - kk,
                            [[AF, 128], [248, segb], [1, kk]]),
                        0.0)

            def emit_slab(lo, hi):
                yva = ybuf[:, 30 + lo : 30 + hi].rearrange(
                    "(c s k) j -> c s k j", c=2, s=8, k=8)
                for par in (0, 1):
                    for s in range(NSEQ):
                        dst = AP(tensor=y_d,
                                 offset=s * T + 1000 * par + lo - W,
                                 ap=[[2000, 8], [1, hi - lo]])
                        eng = nc.scalar if (s % 2 == 0) else nc.sync
                        eng.dma_start(out=dst, in_=yva[par, s])

            def emit_chain(bi, interleave):
                u0, ublk, s0, segb = blocks[bi]
                for sl in range(segb):
                    seg = s0 + sl
                    t0 = seg * S
                    nc.vector._custom_dve(
                        RM,
                        out=tap(fscr, 1, [[9, 128], [1, S], [0, 31]]),
                        in0=tap(afull, t0 * 31,
                                [[AF, 128], [31, S], [1, 31]]),
                        in1=tap(ybuf, t0, [[YS, 128], [1, S], [1, 31]]),
                    )
                    nc.vector._custom_dve(
                        RN,
                        out=tap(ybuf, 30 + t0, [[YS, 128], [1, S], [0, S]]),
                        in0=tap(E, seg * 64, [[ES, 128], [8, S], [1, S]]),
                        in1=tap(fscr, 1, [[9, 128], [0, S], [1, S]]),
                    )
                    if seg == 77:
                        emit_slab(W, W + 500)
                    if seg == 128:
                        emit_slab(W + 500, W + 850)
                    for th in interleave.get(sl, ()):
                        th()

            # ACT interp for the big blocks (after dfhr for dep tracking)
            for bi in range(len(blocks)):
                if bi not in DVE_BLOCKS:
                    emit_interp_mult(bi)

            # -------- block 0 prefix on DVE --------
            emit_interp_mult(0)
            for ch in emit_frb_dmas(0):
                emit_add(0, ch)
            emit_e(0)
            emit_tri_zero(0)

            # -------- pipelined chain --------
            for bi in range(len(blocks)):
                nxt = bi + 1
                interleave = {}
                if nxt < len(blocks):
                    segb = blocks[bi][3]
                    chunks = emit_frb_dmas(nxt)
                    pos = max(1, int(segb * 0.45))
                    th = []
                    if nxt in DVE_BLOCKS:
                        th.append(lambda b=nxt: emit_interp_mult(b))
                    th += [lambda c=ch: emit_add(nxt, c) for ch in chunks]
                    interleave.setdefault(pos, []).extend(th)
                    p2 = max(pos + 1, int(segb * 0.55))
                    kjs = [(kk, jj) for kk in range(1, S) for jj in range(kk)]
                    span = max(1, segb - 2 - p2)
                    for i, (kk, jj) in enumerate(kjs):
                        p = min(segb - 2, p2 + (i * span) // len(kjs))
                        interleave.setdefault(p, []).append(
                            lambda b=nxt, k=kk, q=jj: emit_e_kj(b, k, q))
                    interleave.setdefault(segb - 1, []).append(
                        lambda b=nxt: emit_tri_zero(b))
                emit_chain(bi, interleave)

            # ---------------- final output DMAs ----------------
            emit_slab(W + 850, W + L)

    nc.compile()
    return nc


def _get_prog():
    global _prog
    if _prog is None:
        _prog = _build_program()
    return _prog


def _host_inputs(x, a):
    import ml_dtypes

    x = np.ascontiguousarray(x, dtype=np.float32)
    a = np.ascontiguousarray(a, dtype=np.float32)
    xp = np.zeros((B, XP_LEN), np.float32)
    xp[:, W:] = x
    af = np.concatenate([a, a[:, -1:, :]], axis=1)   # [B, 201, 31]
    nfr = af.shape[1]
    par = np.arange(128) // 64
    sq = (np.arange(128) % 64) // 8
    kc = np.arange(128) % 8
    m = 2 * kc + par
    w0 = 1000 * m - W
    n0 = np.floor_divide(w0, 80)
    phi = w0 - 80 * n0              # 0 or 40
    u = np.arange(NU)
    nl = (40 * u[None, :] + phi[:, None]) // 80
    idx = np.clip(n0[:, None] + nl, 0, nfr - 1)
    idx1 = np.clip(n0[:, None] + nl + 1, 0, nfr - 1)
    jl = np.arange(WP)
    ftab = (((jl[None, :] + phi[:, None]) % 80) / 80.0).astype(np.float32)
    rr = np.arange(80)
    ftabp = (((rr[None, :] + phi[:, None]) % 80) / 80.0).astype(np.float32)
    rev = 30 - np.arange(30)
    in_maps = []
    for c in range(NCORE):
        sl = slice(c * NSEQ, (c + 1) * NSEQ)
        seqg = c * NSEQ + sq
        frhr = af[seqg[:, None, None], idx[:, :, None], rev[None, None, :]]
        frh1r = af[seqg[:, None, None], idx1[:, :, None], rev[None, None, :]]
        kfr = af[seqg[:, None], idx, 0]
        kfr1 = af[seqg[:, None], idx1, 0]
        in_maps.append({
            "xp": xp[sl],
            "frhr": np.ascontiguousarray(frhr, np.float32),
            "frh1r": np.ascontiguousarray(frh1r, np.float32),
            "kfr": np.ascontiguousarray(kfr, np.float32),
            "kfr1": np.ascontiguousarray(kfr1, np.float32),
            "ftab": ftab,
            "ftabp": ftabp,
            "frhrep": np.ascontiguousarray(
                np.repeat(frhr, 40, axis=1).astype(
                    ml_dtypes.bfloat16)).view(np.uint16),
        })
    return in_maps


def kernel(x, a):
    from concourse import bass_utils

    nc = _get_prog()
    in_maps = _host_inputs(x, a)
    res = bass_utils.run_bass_kernel_spmd(nc, in_maps, core_ids=list(range(NCORE)))
    out = np.empty((B, T), np.float32)
    for c in range(NCORE):
        out[c * NSEQ : (c + 1) * NSEQ] = res.results[c]["y"]
    return out
